# revision 26
# baseline (speedup 1.0000x reference)
"""Trainium2 Bass kernel for nn_ConvFrameMaskDecoder.

Self-contained: hardcodes shapes. kernel(**inputs) takes FULL inputs
(as produced by setup_inputs()) and returns the FULL outputs matching
reference(): (actions [B,T,322], scores [B,T,L,1], h [B,512], c [B,512]).

Sharding: pure data parallel, batch B=32 split over 8 cores (4 rows each).
Per core:
  Setup phase (batched over all 192 frames):
    conv1/conv2 (BN-folded 1x1 convs) -> fc (pixel-accumulated) -> visT
    preT_g  = Wv @ visT   (vis contribution to LSTM gates, feature-major)
    pre_act = visT.T @ Av.T (vis contribution to actor logits, batch-major)
    encT / encWT = hfc_w @ encT (attention keys with hfc folded into enc)
  Recurrent loop (T=48), state kept feature-major ([128, chunk*4+b]):
    scores: cross-batch matmul + additive mask + softmax
    weighted: encT-stationary matmuls consuming transposed probs
    gates+actor: weights-moving matmuls, PE-transpose to feature-major
    LSTM cell elementwise in feature-major layout
    greedy decode: segmented max -> one-hot -> fused (table@adapter) matmul
"""

import numpy as np
from contextlib import ExitStack

import concourse.bass as bass
import concourse.bacc as bacc
import concourse.tile as tile
from concourse import mybir
from concourse.bass_utils import run_bass_kernel_spmd
from concourse.masks import make_identity

F32 = mybir.dt.float32
F32R = mybir.dt.float32r
AF = mybir.ActivationFunctionType
ALU = mybir.AluOpType
AX = mybir.AxisListType

NCORES = 8
B, T, L = 32, 48, 64
BL = B // NCORES            # 4 batch rows per core
DH, DF, DE = 512, 512, 128
MC, NB, GD, AD = 4, 52, 3, 9
AOUT = MC + 6 * NB + 2 * GD  # 322
NG = 4 * DH                  # 2048
EPS = 1e-5
NFR = BL * T                 # 192 frames per core
PIX = 49
C1I, C1O, C2O = 512, 256, 64
FLAT = C2O * PIX             # 3136

# frame-group tiling for the conv stage: 16 groups x 3 timesteps x 4 batch
NGRP = 16
TPG = T // NGRP              # 3
FPG = TPG * BL               # 12 frames per group
NPG = FPG * PIX              # 588 columns per group
NSUB = 2                     # conv N split per group
NSL = NPG // NSUB            # 294

# grid for greedy decode: 9 slots x 64 padded classes
GRID = 64
NSLOT = 9
GPAD = 640                   # 5 chunks of 128

_PROG_CACHE = {}


def _prep_weights(params):
    """Host-side preprocessing: fold BN, fuse tables, pre-transpose."""
    p = {k: np.asarray(v, np.float32) for k, v in params.items()}
    out = {}

    s1 = p['bn1_g'] / np.sqrt(p['bn1_v'] + EPS)
    w1 = p['conv1_w'] * s1[:, None]                    # [256, 512]
    b1 = s1 * (p['conv1_b'] - p['bn1_m']) + p['bn1_b']  # [256]
    s2 = p['bn2_g'] / np.sqrt(p['bn2_v'] + EPS)
    w2 = p['conv2_w'] * s2[:, None]                    # [64, 256]
    b2 = s2 * (p['conv2_b'] - p['bn2_m']) + p['bn2_b']  # [64]
    out['cw1T'] = np.ascontiguousarray(w1.T)           # [512, 256]
    out['b1'] = b1
    out['cw2T'] = np.ascontiguousarray(w2.T)           # [256, 64]
    out['b2'] = b2

    # fc: vis = flat @ fc_w.T + fc_b ; flat[c*49+x] -> pixel-accumulated lhsT
    fcT = p['fc_w'].reshape(DF, C2O, PIX)              # [512, 64, 49]
    out['fcT'] = np.ascontiguousarray(fcT.transpose(2, 1, 0))  # [49, 64, 512]
    out['fcb'] = p['fc_b']

    w_ih, w_hh = p['w_ih'], p['w_hh']                  # [2048, 1152], [2048, 512]
    Wv = w_ih[:, 0:DH]                                 # vis part
    Wa = w_ih[:, DH:2 * DH]                            # weighted part
    We = w_ih[:, 2 * DH:2 * DH + DE]                   # e part
    out['PgT'] = np.ascontiguousarray(Wv.T)            # [512, 2048]
    out['bg'] = p['b_ih'] + p['b_hh']                  # [2048]

    actor = p['actor_w']                               # [322, 1664] cols [h|vis|w|e]
    Ah = actor[:, 0:DH]
    Av = actor[:, DH:2 * DH]
    Aw = actor[:, 2 * DH:3 * DH]
    Ae = actor[:, 3 * DH:3 * DH + DE]
    out['AvT'] = np.ascontiguousarray(Av.T)            # [512, 322]
    out['bact'] = p['actor_b']

    # recurrent input order: [weighted(512) | e(128) | h(512)]
    Wg = np.concatenate([Wa, We, w_hh], axis=1)        # [2048, 1152]
    Awe = np.concatenate([Aw, Ae], axis=1)             # [322, 640]
    Wbig = np.zeros((NG + AOUT, 1152), np.float32)
    Wbig[:NG] = Wg
    Wbig[NG:, :DH + DE] = Awe
    out['WgT'] = np.ascontiguousarray(Wbig.T)          # [1152, 2370]
    out['AhT'] = np.ascontiguousarray(Ah.T)            # [512, 322]

    # score[b,l] = sum_i h[b,i] * (sum_d enc[b,l,d] hfc_w[d,i]) -> fold as
    # lhsT[K=d, M=i] = hfc_w (row index d is hfc_w's leading/output dim)
    out['hfcwT'] = np.ascontiguousarray(p['hfc_w'])    # [512, 512]
    out['hfcb'] = p['hfc_b']                           # [512]

    # fused decode tables: grid row 64*s + j -> table_s[j] @ adapter_s.T
    A = p['adapter_w']                                 # [128, 9*128]
    tables = [p['emb_mode'], p['emb_xy'], p['emb_xy'], p['emb_yaw'],
              p['emb_eff_xyz'], p['emb_eff_xyz'], p['emb_eff_xyz'],
              p['emb_grasp_drop'], p['emb_up_down']]
    fused = np.zeros((GPAD, DE), np.float32)
    for s, tab in enumerate(tables):
        As = A[:, s * DE:(s + 1) * DE]                 # [128, 128]
        fused[GRID * s:GRID * s + tab.shape[0]] = tab @ As.T
    out['fusedT'] = fused                              # [640, 128]
    out['adb'] = p['adapter_b']                        # [128]
    out['go'] = p['go']                                # [128]
    return {k: np.ascontiguousarray(np.asarray(v, np.float32)) for k, v in out.items()}


def _build_program(debug=False):
    nc = bacc.Bacc()
    dp = nc.declare_dram_parameter
    d_dbg = {}
    DBG_T = (0, 1)
    if debug:
        for nm, shape in [("visT", [128, 4, NFR]), ("preTg", [128, 16, NFR]),
                          ("encWT", [128, 4, BL * L])]:
            d_dbg[nm] = dp("dbg_" + nm, shape, F32, isOutput=True)
        for tt in DBG_T:
            for nm, shape in [("wT", [128, 16]), ("g_s", [BL, NG]),
                              ("gTs", [128, 64]), ("hT1", [128, 16]),
                              ("cT1", [128, 16]), ("ohg", [BL, GPAD]),
                              ("eT1", [128, BL]), ("probs0", [BL, BL * L]),
                              ("ms0", [BL, BL * L])]:
                d_dbg[f"{nm}@{tt}"] = dp(f"dbg_{nm}_{tt}", shape, F32, isOutput=True)

    d_frames = dp("frames", [NGRP, 128, 4, NPG], F32, isOutput=False)
    d_enc = dp("enc", [BL, L, DH], F32, isOutput=False)
    d_h0 = dp("h0", [BL, DH], F32, isOutput=False)
    d_c0 = dp("c0", [BL, DH], F32, isOutput=False)

    d_cw1T = dp("cw1T", [C1I, C1O], F32, isOutput=False)
    d_b1 = dp("b1", [C1O], F32, isOutput=False)
    d_cw2T = dp("cw2T", [C1O, C2O], F32, isOutput=False)
    d_b2 = dp("b2", [C2O], F32, isOutput=False)
    d_fcT = dp("fcT", [PIX, C2O, DF], F32, isOutput=False)
    d_fcb = dp("fcb", [DF], F32, isOutput=False)
    d_PgT = dp("PgT", [DH, NG], F32, isOutput=False)
    d_bg = dp("bg", [NG], F32, isOutput=False)
    d_AvT = dp("AvT", [DH, AOUT], F32, isOutput=False)
    d_bact = dp("bact", [AOUT], F32, isOutput=False)
    d_WgT = dp("WgT", [DH + DE + DH, NG + AOUT], F32, isOutput=False)
    d_AhT = dp("AhT", [DH, AOUT], F32, isOutput=False)
    d_hfcwT = dp("hfcwT", [DH, DH], F32, isOutput=False)
    d_hfcb = dp("hfcb", [DH], F32, isOutput=False)
    d_fusedT = dp("fusedT", [GPAD, DE], F32, isOutput=False)
    d_adb = dp("adb", [DE], F32, isOutput=False)
    d_go = dp("go", [DE], F32, isOutput=False)

    d_actions = dp("actions", [BL, T, AOUT], F32, isOutput=True)
    d_scoresT = dp("scoresT_out", [128, T, 8], F32, isOutput=True)
    d_hout = dp("h_out", [BL, DH], F32, isOutput=True)
    d_cout = dp("c_out", [BL, DH], F32, isOutput=True)

    with tile.TileContext(nc) as tc, ExitStack() as ctx:
        persist = ctx.enter_context(tc.tile_pool(name="persist", bufs=1))

        # ---------------- persistent small tensors ----------------
        ident = persist.tile([128, 128], F32)
        make_identity(nc, ident)
        ones_row = persist.tile([1, 128], F32)
        nc.vector.memset(ones_row, 1.0)

        # additive mask for cross-batch attention scores: [4, 4*64]
        mask = persist.tile([BL, BL * L], F32)
        nc.vector.memset(mask, 0.0)
        nc.gpsimd.affine_select(
            out=mask, in_=mask, pattern=[[1, BL], [0, L]],
            compare_op=ALU.is_equal, fill=-1e30, base=0, channel_multiplier=-1)

        AhT = persist.tile([128, 4, AOUT], F32)
        nc.sync.dma_start(out=AhT, in_=d_AhT[:].rearrange("(c p) n -> p c n", p=128))
        fusedT = persist.tile([128, 5, DE], F32)
        nc.sync.dma_start(out=fusedT, in_=d_fusedT[:].rearrange("(q p) e -> p q e", p=128))
        adbT = persist.tile([128, 1], F32)
        nc.sync.dma_start(out=adbT, in_=d_adb[:].unsqueeze(1))

        enc_bm = persist.tile([128, 2, DH], F32)   # [(b%2)*64+l, b//2, d]
        for q in range(2):
            nc.sync.dma_start(
                out=enc_bm[:, q, :],
                in_=d_enc[2 * q:2 * q + 2].rearrange("f l d -> (f l) d"))

        encWT = persist.tile([128, 4, BL * L], F32)
        encb_s = persist.tile([1, BL * L], F32)
        preTg = persist.tile([128, 16, NFR], F32)
        scoresT = persist.tile([128, T, 8], F32)
        AvT = persist.tile([128, 4, AOUT], F32)
        nc.sync.dma_start(out=AvT, in_=d_AvT[:].rearrange("(c p) n -> p c n", p=128))
        bact_r = persist.tile([1, AOUT], F32)
        nc.sync.dma_start(out=bact_r, in_=d_bact[:].unsqueeze(0))

        # recurrent state (feature-major): col index = chunk*4 + b
        hT = persist.tile([128, 4, BL], F32)
        cT = persist.tile([128, 4 * BL], F32)
        eT = persist.tile([128, BL], F32)
        go_s = persist.tile([128, 1], F32)
        nc.sync.dma_start(out=go_s, in_=d_go[:].unsqueeze(1))
        nc.vector.tensor_copy(eT, go_s.to_broadcast([128, BL]))

        oh_grid = persist.tile([BL, GPAD], F32)
        nc.vector.memset(oh_grid, 0.0)

        visT = persist.tile([128, 4, NFR], F32)

        # ---------------- setup phase B pool (opened first, survives A) ----
        setupB = ctx.enter_context(tc.tile_pool(name="setupB", bufs=1))
        PgT = setupB.tile([128, 4, NG], F32)
        nc.sync.dma_start(out=PgT, in_=d_PgT[:].rearrange("(c p) n -> p c n", p=128))
        bgT = setupB.tile([128, 16], F32)
        nc.sync.dma_start(out=bgT, in_=d_bg[:].rearrange("(c p) -> p c", p=128))
        hfcwT = setupB.tile([128, 4, DH], F32)
        nc.sync.dma_start(out=hfcwT, in_=d_hfcwT[:].rearrange("(c p) n -> p c n", p=128))
        hfcbT = setupB.tile([128, 4], F32)
        nc.sync.dma_start(out=hfcbT, in_=d_hfcb[:].rearrange("(c p) -> p c", p=128))
        fcbT = setupB.tile([128, 4], F32)
        nc.sync.dma_start(out=fcbT, in_=d_fcb[:].rearrange("(c p) -> p c", p=128))

        # ---------------- setup phase A: conv1/conv2/fc -------------------
        with tc.tile_pool(name="setupA", bufs=1) as sa, \
             tc.tile_pool(name="psA", bufs=1, space="PSUM") as psA:
            cw1T = sa.tile([128, 4, C1O], F32)
            nc.sync.dma_start(out=cw1T, in_=d_cw1T[:].rearrange("(c p) m -> p c m", p=128))
            b1T = sa.tile([128, 2], F32)
            nc.sync.dma_start(out=b1T, in_=d_b1[:].rearrange("(c p) -> p c", p=128))
            cw2T = sa.tile([128, 2, C2O], F32)
            nc.sync.dma_start(out=cw2T, in_=d_cw2T[:].rearrange("(c p) m -> p c m", p=128))
            b2T = sa.tile([64, 1], F32)
            nc.sync.dma_start(out=b2T, in_=d_b2[:].unsqueeze(1))

            vis2 = sa.tile([64, NGRP, NPG], F32)

            for g in range(NGRP):
                X = sa.tile([128, 4, NPG], F32, tag="X", bufs=2)
                nc.sync.dma_start(out=X, in_=d_frames[g])
                out1 = sa.tile([128, 2, NPG], F32, tag="out1", bufs=2)
                for m in range(2):
                    for ns in range(NSUB):
                        ps1 = psA.tile([128, NSL], F32, tag="ps1", bufs=2)
                        for k in range(4):
                            nc.tensor.matmul(
                                ps1,
                                cw1T[:, k, 128 * m:128 * (m + 1)],
                                X[:, k, NSL * ns:NSL * (ns + 1)],
                                start=(k == 0), stop=(k == 3))
                        dst = out1[:, m, NSL * ns:NSL * (ns + 1)]
                        if ns == 0:
                            nc.scalar.activation(dst, ps1, AF.Relu,
                                                 bias=b1T[:, m:m + 1], scale=1.0)
                        else:
                            nc.vector.tensor_scalar(dst, ps1, b1T[:, m:m + 1], 0.0,
                                                    ALU.add, ALU.max)
                for ns in range(NSUB):
                    ps2 = psA.tile([64, NSL], F32, tag="ps2", bufs=2)
                    for k in range(2):
                        nc.tensor.matmul(
                            ps2, cw2T[:, k, :],
                            out1[:, k, NSL * ns:NSL * (ns + 1)],
                            start=(k == 0), stop=(k == 1))
                    dst = vis2[:, g, NSL * ns:NSL * (ns + 1)]
                    if ns == 0:
                        nc.scalar.activation(dst, ps2, AF.Relu,
                                             bias=b2T, scale=1.0)
                    else:
                        nc.vector.tensor_scalar(dst, ps2, b2T, 0.0, ALU.add, ALU.max)

            # fc: visT[128, m, f] ; rhs = vis2 pixel slices [64, NGRP, FPG]
            for m in range(4):
                fcTs = sa.tile([64, PIX, 128], F32, tag="fcTs", bufs=1)
                nc.sync.dma_start(
                    out=fcTs,
                    in_=d_fcT[:, :, 128 * m:128 * (m + 1)].rearrange("x c q -> c x q"))
                psf = psA.tile([128, NFR], F32, tag="psf", bufs=2)
                for x in range(PIX):
                    rhs = vis2.rearrange("c g (f x) -> c x (g f)", x=PIX)[:, x, :]
                    nc.tensor.matmul(psf, fcTs[:, x, :],
                                     rhs,
                                     start=(x == 0), stop=(x == PIX - 1))
                nc.scalar.activation(visT[:, m, :], psf, AF.Identity,
                                     bias=fcbT[:, m:m + 1], scale=1.0)

        # ---------------- WgT load (after setupA freed) -------------------
        wpool = ctx.enter_context(tc.tile_pool(name="wpool", bufs=1))
        WgT = wpool.tile([128, 9, NG + AOUT], F32)
        nc.sync.dma_start(out=WgT, in_=d_WgT[:].rearrange("(k p) n -> p k n", p=128))

        # ---------------- setup phase B compute ---------------------------
        with tc.tile_pool(name="psB", bufs=1, space="PSUM") as psB:
            # h0/c0 -> feature-major state via PE transpose
            h0_s = setupB.tile([BL, DH], F32)
            nc.sync.dma_start(out=h0_s, in_=d_h0[:])
            c0_s = setupB.tile([BL, DH], F32)
            nc.sync.dma_start(out=c0_s, in_=d_c0[:])
            for src, dst in ((h0_s, hT.rearrange("p c b -> p (c b)")), (c0_s, cT)):
                pst = psB.tile([128, 4 * BL], F32, tag="pse", bufs=2, name="pst")
                for c in range(4):
                    nc.tensor.transpose(pst[:, 4 * c:4 * c + 4],
                                        src[:, 128 * c:128 * (c + 1)], ident[:BL, :BL])
                nc.vector.tensor_copy(dst, pst)
            # preT_g [128, 16, 192]
            for m in range(16):
                psg = psB.tile([128, NFR], F32, tag="psg", bufs=2)
                for c in range(4):
                    nc.tensor.matmul(psg, PgT[:, c, 128 * m:128 * (m + 1)],
                                     visT[:, c, :],
                                     start=(c == 0), stop=(c == 3))
                nc.scalar.activation(preTg[:, m, :], psg, AF.Identity,
                                     bias=bgT[:, m:m + 1], scale=1.0)
            # encT via PE transpose; then encWT = hfc_w @ encT, encb = hfc_b @ encT
            encT = setupB.tile([128, 4, 2 * 128], F32)
            for c in range(4):
                pse = psB.tile([128, 2 * 128], F32, tag="pse", bufs=2)
                for q in range(2):
                    nc.tensor.transpose(pse[:, 128 * q:128 * (q + 1)],
                                        enc_bm[:, q, 128 * c:128 * (c + 1)],
                                        ident)
                nc.vector.tensor_copy(encT[:, c, :], pse)
            for m in range(4):
                psw = psB.tile([128, BL * L], F32, tag="pse", bufs=2)
                for c in range(4):
                    nc.tensor.matmul(psw, hfcwT[:, c, 128 * m:128 * (m + 1)],
                                     encT[:, c, :],
                                     start=(c == 0), stop=(c == 3))
                nc.vector.tensor_copy(encWT[:, m, :], psw)
            psb2 = psB.tile([1, BL * L], F32, tag="psb2", bufs=1)
            for c in range(4):
                nc.tensor.matmul(psb2, hfcbT[:, c:c + 1],
                                 encT[:, c, :],
                                 start=(c == 0), stop=(c == 3))
            nc.vector.tensor_copy(encb_s, psb2)

        # ---------------- recurrent loop ----------------------------------
        lp = ctx.enter_context(tc.tile_pool(name="lp", bufs=1))
        loop_ps = ctx.enter_context(tc.tile_pool(name="loop_ps", bufs=1, space="PSUM"))

        for t in range(T):
            # ---- attention scores (cross-batch trick + additive mask)
            p_sc = loop_ps.tile([BL, BL * L], F32, tag="sm", bufs=3, name="p_sc")
            for c in range(4):
                nc.tensor.matmul(p_sc, hT[:, c, :],
                                 encWT[:, c, :],
                                 start=(c == 0), stop=False)
            nc.tensor.matmul(p_sc, ones_row[:, 0:BL],
                             encb_s, start=False, stop=True)
            ms = lp.tile([BL, BL * L], F32, tag="ms", bufs=2, name="ms")
            nc.vector.tensor_add(ms, p_sc, mask)
            nm = lp.tile([BL, 1], F32, tag="nm", bufs=2, name="nm")
            nc.vector.tensor_reduce(nm, ms, axis=AX.X, op=ALU.max, negate=True)
            probs = lp.tile([BL, BL * L], F32, tag="probs", bufs=2, name="probs")
            se = lp.tile([BL, 1], F32, tag="se", bufs=2, name="se")
            nc.scalar.activation(probs, ms, AF.Exp, bias=nm, scale=1.0, accum_out=se)
            rc = lp.tile([BL, 1], F32, tag="rc", bufs=2, name="rc")
            nc.vector.reciprocal(rc, se)
            nc.vector.tensor_scalar_mul(probs, probs, rc)
            # transpose probs -> scoresT[:, t, :]
            p_sT = loop_ps.tile([128, 8], F32, tag="sm", bufs=3, name="p_sT")
            for q in range(2):
                nc.tensor.transpose(p_sT[:, 4 * q:4 * q + 4],
                                    probs[:, 128 * q:128 * (q + 1)], ident[:BL, :BL])
            nc.vector.tensor_copy(scoresT[:, t, :], p_sT)

            # ---- weighted (feature-major): enc stationary, probsT moving
            p_wT = loop_ps.tile([128, 4 * BL], F32, tag="sm", bufs=3, name="p_wT")
            for m in range(4):
                for q in range(2):
                    nc.tensor.matmul(p_wT[:, 4 * m:4 * m + 4],
                                     enc_bm[:, q, 128 * m:128 * (m + 1)],
                                     scoresT[:, t, 4 * q:4 * q + 4],
                                     start=(q == 0), stop=(q == 1))
            wT = lp.tile([128, 4 * BL], F32, tag="wT", bufs=2, name="wT")
            nc.vector.tensor_copy(wT, p_wT)

            # ---- gates + actor(w,e part): weights moving
            p_g = loop_ps.tile([BL, NG], F32, tag="g", bufs=1, name="p_g")
            p_a = loop_ps.tile([BL, AOUT], F32, tag="a", bufs=1, name="p_a")
            for c in range(4):
                nc.tensor.matmul(p_a, visT[:, c, BL * t:BL * (t + 1)],
                                 AvT[:, c, :],
                                 start=(c == 0), stop=False)
            nc.tensor.matmul(p_a, ones_row[:, 0:BL],
                             bact_r, start=False, stop=False)
            wT4 = wT.rearrange("p (c b) -> p c b", b=BL)
            for k in range(9):
                if k < 4:
                    lhs = wT4[:, k, :]
                elif k == 4:
                    lhs = eT
                else:
                    lhs = hT[:, k - 5, :]
                lhs = lhs
                for j in range(4):
                    nc.tensor.matmul(p_g[:, 512 * j:512 * (j + 1)], lhs,
                                     WgT[:, k, 512 * j:512 * (j + 1)],
                                     start=(k == 0), stop=(k == 8))
                if k < 5:
                    nc.tensor.matmul(p_a, lhs, WgT[:, k, NG:],
                                     start=False, stop=False)

            # ---- evacuate gates (split ACT/DVE), transpose to feature-major
            g_s = lp.tile([BL, NG], F32, tag="g_s", bufs=1, name="g_s")
            nc.scalar.copy(g_s[:, 0:1024], p_g[:, 0:1024])
            nc.vector.tensor_copy(g_s[:, 1024:2048], p_g[:, 1024:2048])
            p_gT = loop_ps.tile([128, 64], F32, tag="sm", bufs=3, name="p_gT")
            for c in range(16):
                nc.tensor.transpose(p_gT[:, 4 * c:4 * c + 4],
                                    g_s[:, 128 * c:128 * (c + 1)], ident[:BL, :BL])
            gTs = lp.tile([128, 64], F32, tag="gTs", bufs=2, name="gTs")
            nc.vector.tensor_add(gTs, p_gT, preTg[:, :, BL * t:BL * (t + 1)])

            # ---- LSTM cell (feature-major, 16 cols per gate)
            sg = lp.tile([128, 64], F32, tag="sg", bufs=2, name="sg")
            nc.scalar.activation(sg[:, 0:32], gTs[:, 0:32], AF.Sigmoid)
            nc.scalar.activation(sg[:, 48:64], gTs[:, 48:64], AF.Sigmoid)
            tg = lp.tile([128, 16], F32, tag="tg", bufs=2, name="tg")
            nc.scalar.activation(tg, gTs[:, 32:48], AF.Tanh)
            tmp1 = lp.tile([128, 16], F32, tag="tmp1", bufs=2, name="tmp1")
            nc.vector.tensor_mul(tmp1, sg[:, 16:32], cT)
            tmp2 = lp.tile([128, 16], F32, tag="tmp2", bufs=2, name="tmp2")
            nc.vector.tensor_mul(tmp2, sg[:, 0:16], tg)
            nc.vector.tensor_add(cT, tmp1, tmp2)
            tcn = lp.tile([128, 16], F32, tag="tcn", bufs=2, name="tcn")
            nc.scalar.activation(tcn, cT, AF.Tanh)
            nc.vector.tensor_mul(hT.rearrange("p c b -> p (c b)"), sg[:, 48:64], tcn)

            # ---- actor h-part (new h)
            for c in range(4):
                nc.tensor.matmul(p_a, hT[:, c, :],
                                 AhT[:, c, :],
                                 start=False, stop=(c == 3))

            # ---- emit action logits (recurrent part + vis/bias pre part)
            a_s = lp.tile([BL, AOUT], F32, tag="a_s", bufs=2, name="a_s")
            nc.scalar.copy(a_s, p_a)
            nc.sync.dma_start(out=d_actions[:, t, :], in_=a_s)

            if debug and t in DBG_T:
                for nm, dsrc in [("wT", wT), ("g_s", g_s), ("gTs", gTs),
                                 ("hT1", hT.rearrange("p c b -> p (c b)")),
                                 ("cT1", cT), ("probs0", probs), ("ms0", ms)]:
                    nc.sync.dma_start(out=d_dbg[f"{nm}@{t}"][:], in_=dsrc)

            if t == T - 1:
                continue

            # ---- greedy decode -> one-hot grid (batch-major)
            mx = lp.tile([BL, NSLOT], F32, tag="mx", bufs=2, name="mx")
            nc.vector.tensor_reduce(mx[:, 0:1], a_s[:, 0:MC], axis=AX.X, op=ALU.max)
            nc.vector.tensor_reduce(
                mx[:, 1:7], a_s[:, MC:MC + 6 * NB].rearrange("p (s n) -> p s n", s=6),
                axis=AX.X, op=ALU.max)
            nc.vector.tensor_reduce(
                mx[:, 7:9], a_s[:, MC + 6 * NB:].rearrange("p (s n) -> p s n", s=2),
                axis=AX.X, op=ALU.max)
            nc.vector.tensor_tensor(
                oh_grid[:, 0:MC], a_s[:, 0:MC],
                mx[:, 0:1].to_broadcast([BL, MC]), ALU.is_equal)
            nc.vector.tensor_tensor(
                oh_grid[:, GRID:GRID * 7].rearrange("p (s n) -> p s n", n=GRID)[:, :, 0:NB],
                a_s[:, MC:MC + 6 * NB].rearrange("p (s n) -> p s n", s=6),
                mx[:, 1:7].unsqueeze(2).to_broadcast([BL, 6, NB]), ALU.is_equal)
            nc.vector.tensor_tensor(
                oh_grid[:, GRID * 7:GRID * 9].rearrange("p (s n) -> p s n", n=GRID)[:, :, 0:GD],
                a_s[:, MC + 6 * NB:].rearrange("p (s n) -> p s n", s=2),
                mx[:, 7:9].unsqueeze(2).to_broadcast([BL, 2, GD]), ALU.is_equal)

            # ---- one-hot -> fused embedding+adapter -> new eT
            p_oT = loop_ps.tile([128, 20], F32, tag="sm", bufs=3, name="p_oT")
            for q in range(5):
                nc.tensor.transpose(p_oT[:, 4 * q:4 * q + 4],
                                    oh_grid[:, 128 * q:128 * (q + 1)], ident[:BL, :BL])
            oT = lp.tile([128, 20], F32, tag="oT", bufs=2, name="oT")
            nc.vector.tensor_copy(oT, p_oT)
            p_e = loop_ps.tile([128, BL], F32, tag="sm", bufs=3, name="p_e")
            for q in range(5):
                nc.tensor.matmul(p_e, fusedT[:, q, :],
                                 oT[:, 4 * q:4 * q + 4],
                                 start=(q == 0), stop=(q == 4))
            nc.scalar.activation(eT, p_e, AF.Identity, bias=adbT, scale=1.0)

            if debug and t in DBG_T:
                nc.sync.dma_start(out=d_dbg[f"ohg@{t}"][:], in_=oh_grid)
                nc.sync.dma_start(out=d_dbg[f"eT1@{t}"][:], in_=eT)
            if debug and t == 0:
                nc.sync.dma_start(out=d_dbg["visT"][:], in_=visT)
                nc.sync.dma_start(out=d_dbg["preTg"][:], in_=preTg)
                nc.sync.dma_start(out=d_dbg["encWT"][:], in_=encWT)

        # ---------------- final outputs -----------------------------------
        for src, dst in ((hT.rearrange("p c b -> p (c b)"), d_hout), (cT, d_cout)):
            p_hb = loop_ps.tile([BL, DH], F32, tag="sm", bufs=3, name="p_hb")
            src4 = src.rearrange("p (c b) -> p c b", b=BL)
            for c in range(4):
                nc.tensor.transpose(p_hb[:, 128 * c:128 * (c + 1)], src4[:, c, :],
                                    ident)
            hb_s = lp.tile([BL, DH], F32, tag="hb_s", bufs=2, name="hb_s")
            nc.vector.tensor_copy(hb_s, p_hb)
            nc.sync.dma_start(out=dst[:], in_=hb_s)

        nc.sync.dma_start(out=d_scoresT[:], in_=scoresT)

    if not nc.is_finalized():
        nc.finalize()
    return nc


def _get_program():
    if 'nc' not in _PROG_CACHE:
        _PROG_CACHE['nc'] = _build_program()
    return _PROG_CACHE['nc']


def frames_to_X(frames_core):
    """[BL, T, 512, 49] -> [NGRP, 128, 4, NPG] in (g, p, k, (tau b x)) order."""
    fx = frames_core.reshape(BL, NGRP, TPG, 4, 128, PIX)
    fx = fx.transpose(1, 4, 3, 2, 0, 5)   # g, p, k, tau, b, x
    return np.ascontiguousarray(fx.reshape(NGRP, 128, 4, NPG))


def scores_from_T(st):
    """[128, T, 8] transposed per-step score tile -> [BL, T, L]."""
    out = np.empty((BL, T, L), np.float32)
    for b in range(BL):
        out[b] = st[64 * (b % 2):64 * (b % 2) + 64, :, 4 * (b // 2) + b].T
    return out


def _ensure_ntff_hook():
    """Inject antenv.axon_hooks (absent in this image) so trace=True works."""
    try:
        from antenv.axon_hooks import get_axon_ntff_profile_hook  # noqa: F401
        return
    except ImportError:
        pass
    import sys
    import types
    try:
        import antenv
        from trn_agent_boot.trn_boot import _ntff_profile_via_ctypes
    except ImportError:
        return
    mod = types.ModuleType("antenv.axon_hooks")
    state = {}
    mod.set_axon_ntff_profile_hook = lambda h: state.__setitem__('h', h)
    mod.get_axon_ntff_profile_hook = lambda: state.get('h')
    sys.modules['antenv.axon_hooks'] = mod
    antenv.axon_hooks = mod
    try:
        hook = _ntff_profile_via_ctypes('/opt/axon/libaxon_pjrt.so')
    except OSError:
        hook = None
    if hook is not None:
        mod.set_axon_ntff_profile_hook(hook)


def run_kernel(inputs, trace=False):
    if trace:
        _ensure_ntff_hook()
    nc = _get_program()
    prep = _prep_weights(inputs['params'])
    frames = np.ascontiguousarray(
        np.asarray(inputs['frames'], np.float32).reshape(B, T, C1I, PIX))
    enc = np.asarray(inputs['enc'], np.float32)
    h0 = np.asarray(inputs['h0'], np.float32)
    c0 = np.asarray(inputs['c0'], np.float32)

    in_maps = []
    for c in range(NCORES):
        sl = slice(BL * c, BL * (c + 1))
        m = dict(prep)
        m['frames'] = frames_to_X(frames[sl])
        m['enc'] = np.ascontiguousarray(enc[sl])
        m['h0'] = np.ascontiguousarray(h0[sl])
        m['c0'] = np.ascontiguousarray(c0[sl])
        in_maps.append(m)

    res = run_bass_kernel_spmd(nc, in_maps, list(range(NCORES)), trace=trace)
    actions = np.concatenate([r['actions'] for r in res.results], axis=0)
    scores = np.concatenate(
        [scores_from_T(r['scoresT_out']) for r in res.results], axis=0)[..., None]
    h = np.concatenate([r['h_out'] for r in res.results], axis=0)
    c = np.concatenate([r['c_out'] for r in res.results], axis=0)
    return (actions, scores, h, c), res


def kernel(**inputs):
    (actions, scores, h, c), _ = run_kernel(inputs)
    return actions, scores, h, c
